# revision 6
# baseline (speedup 1.0000x reference)
"""Trainium2 Bass kernel for nn_DirectionalConv (moe_routing).

Math: out = (1/8) * sum_k conv3x3(x * [octant(sobel(x)) == k], W[k]) + mean_k b[k]

Implementation notes:
- Data-parallel over batch B=8 across 8 NeuronCores (one image per core).
- Octant selection is rewritten in a +-1 "monomial" basis over the three sign
  bits (sign(gy), sign(gx), sign(|gy|-|gx|)):
      sum_k conv(x*mask_k, W[k]) = sum_{S in 2^3} conv(x*chi_S, W'_S)
  where chi_S = product of the selected signs (computed with pure bitwise
  XOR of sign bits - exact) and W'_S = (1/64) sum_k chi_S(k) W[k] is
  precomputed on the host.  This gives 8 dense 3x3 convs, evaluated as
  9 shifted matmuls each, accumulating in PSUM.
- Per-core image (64,256,256) is split into top/bottom halves across the
  SBUF partition dimension: partition p = (half<<6)|channel.  Conv matmuls
  are K=64 and use 4-way PE tile packing (2 row-groups x 2 col-groups) to
  fill the 128x128 array.
- Sobel gradients are computed in fp32 on the vector engine (separable,
  scalar_tensor_tensor fused ops); monomials/weights are fp16 (rel err ~1e-3).
"""

import numpy as np

import concourse.bacc as bacc
import concourse.bass as bass
import concourse.mybir as mybir
from concourse import bass_utils
from concourse.tile import TileContext

F32 = mybir.dt.float32
F16 = mybir.dt.float16
U32 = mybir.dt.uint32
ALU = mybir.AluOpType
ACTF = mybir.ActivationFunctionType

B, C, H, W_, K, O = 8, 64, 256, 256, 8, 64
HH = H // 2          # rows per half
R = 8                # output rows per half per chunk
NCHUNK = HH // R     # 16
WP = W_ + 2          # padded width 258
SIGN16 = 0x80008000  # sign bits of two packed fp16 lanes


def _build_nc(debug_chunk=None):
    nc = bacc.Bacc("TRN2", target_bir_lowering=False, debug=False)

    x_d = nc.dram_tensor("x", [C, H, W_], F32, kind="ExternalInput")
    wt_d = nc.dram_tensor("wt", [128, 8, 9, O], F16, kind="ExternalInput")
    bias_d = nc.dram_tensor("bias", [128, 1], F32, kind="ExternalInput")
    out_d = nc.dram_tensor("out", [O, H, W_], F32, kind="ExternalOutput")
    if debug_chunk is not None:
        dbg_mono = nc.dram_tensor("dbg_mono", [128, 8, R + 2, WP], F16,
                                  kind="ExternalOutput")
        dbg_g = nc.dram_tensor("dbg_g", [128, 3, R + 2, WP], F16,
                               kind="ExternalOutput")
        dbg_x = nc.dram_tensor("dbg_x", [128, R + 4, WP], F32,
                               kind="ExternalOutput")

    with TileContext(nc) as tc:
        with (
            tc.tile_pool(name="wpool", bufs=1) as wpool,
            tc.tile_pool(name="xpool", bufs=2) as xpool,
            tc.tile_pool(name="tpool", bufs=1) as tpool,
            tc.tile_pool(name="mpool", bufs=2) as mpool,
            tc.tile_pool(name="spool", bufs=4) as spool,
            tc.tile_pool(name="ppool", bufs=2, space="PSUM") as ppool,
        ):
            wt = wpool.tile([128, 8, 9, O], F16)
            nc.sync.dma_start(wt[:], wt_d[:])
            biasT = wpool.tile([128, 1], F32)
            nc.sync.dma_start(biasT[:], bias_d[:])
            maskT = wpool.tile([128, 1], U32)
            nc.gpsimd.memset(maskT[:], SIGN16)

            for ci in range(NCHUNK):
                r0 = ci * R
                # ---- load x chunk (rows r0-2 .. r0+R+2 per half), cols 1..256
                xt = xpool.tile([128, R + 4, WP], F32, tag="xt")
                nc.gpsimd.memset(xt[:, :, 0:1], 0.0)
                nc.gpsimd.memset(xt[:, :, WP - 1:WP], 0.0)
                # top half -> partitions 0..63
                tlo, thi = r0 - 2, r0 + R + 2
                if tlo < 0:
                    nc.gpsimd.memset(xt[0:64, 0:-tlo, 1:WP - 1], 0.0)
                    nc.sync.dma_start(xt[0:64, -tlo:R + 4, 1:WP - 1],
                                      x_d[:, 0:thi, :])
                else:
                    nc.sync.dma_start(xt[0:64, :, 1:WP - 1], x_d[:, tlo:thi, :])
                # bottom half -> partitions 64..127
                blo, bhi = HH + r0 - 2, HH + r0 + R + 2
                if bhi > H:
                    nval = H - blo
                    nc.gpsimd.memset(xt[64:128, nval:R + 4, 1:WP - 1], 0.0)
                    nc.sync.dma_start(xt[64:128, 0:nval, 1:WP - 1],
                                      x_d[:, blo:H, :])
                else:
                    nc.sync.dma_start(xt[64:128, :, 1:WP - 1], x_d[:, blo:bhi, :])

                # ---- Sobel gradients (fp32, separable) on rows r0-1..r0+R
                RG = R + 2
                at = tpool.tile([128, RG, WP], F32, tag="at")
                nc.vector.tensor_add(at[:], xt[:, 0:RG, :], xt[:, 2:RG + 2, :])
                tt = tpool.tile([128, RG, WP], F32, tag="tt")
                nc.vector.scalar_tensor_tensor(tt[:], xt[:, 1:RG + 1, :], 2.0,
                                               at[:], ALU.mult, ALU.add)
                ut = tpool.tile([128, RG, WP], F32, tag="ut")
                nc.vector.tensor_sub(ut[:], xt[:, 0:RG, :], xt[:, 2:RG + 2, :])
                gx32 = tpool.tile([128, RG, WP], F32, tag="gx32")
                nc.gpsimd.memset(gx32[:, :, 0:1], 0.0)
                nc.gpsimd.memset(gx32[:, :, WP - 1:WP], 0.0)
                nc.vector.tensor_tensor(gx32[:, :, 1:WP - 1], tt[:, :, 0:WP - 2],
                                        tt[:, :, 2:WP], ALU.subtract)
                b2 = tpool.tile([128, RG, W_], F32, tag="at")  # reuse at's slot
                nc.vector.tensor_add(b2[:], ut[:, :, 0:WP - 2], ut[:, :, 2:WP])
                gy32 = tpool.tile([128, RG, WP], F32, tag="gy32")
                nc.gpsimd.memset(gy32[:, :, 0:1], 0.0)
                nc.gpsimd.memset(gy32[:, :, WP - 1:WP], 0.0)
                nc.vector.scalar_tensor_tensor(gy32[:, :, 1:WP - 1],
                                               ut[:, :, 1:WP - 1], 2.0, b2[:],
                                               ALU.mult, ALU.add)
                # fp16 sign sources (casts on the scalar engine, which is idle)
                gxh = tpool.tile([128, RG, WP], F16, tag="gxh")
                nc.scalar.activation(gxh[:], gx32[:], ACTF.Copy)
                gyh = tpool.tile([128, RG, WP], F16, tag="gyh")
                nc.scalar.activation(gyh[:], gy32[:], ACTF.Copy)
                # e = |gy|-|gx| in fp32 (only its sign is used; fp16 rounding
                # of the comparison would misbin ~1e-4 of pixels -> 1.5e-2 err)
                ay = tpool.tile([128, RG, W_], F32, tag="tt")
                nc.scalar.activation(ay[:], gy32[:, :, 1:WP - 1], ACTF.Abs)
                ax = tpool.tile([128, RG, W_], F32, tag="ut")
                nc.scalar.activation(ax[:], gx32[:, :, 1:WP - 1], ACTF.Abs)
                e32 = tpool.tile([128, RG, WP], F32, tag="e32")
                nc.gpsimd.memset(e32[:, :, 0:1], 0.0)
                nc.gpsimd.memset(e32[:, :, WP - 1:WP], 0.0)
                nc.vector.tensor_tensor(e32[:, :, 1:WP - 1], ay[:], ax[:],
                                        ALU.subtract)
                eh = tpool.tile([128, RG, WP], F16, tag="eh")
                nc.scalar.activation(eh[:], e32[:], ACTF.Copy)

                # ---- monomials y_S = x * chi_S as fp16, S = (sy<<2)|(sx<<1)|sd
                mono = mpool.tile([128, 8, RG, WP], F16, tag="mono")
                nc.vector.tensor_copy(mono[:, 0], xt[:, 1:RG + 1, :])
                mu = [mono[:, S].bitcast(U32) for S in range(8)]
                sy = gyh[:].bitcast(U32)
                sx = gxh[:].bitcast(U32)
                sd = eh[:].bitcast(U32)
                mk = maskT[:, 0:1]
                stt = nc.vector.scalar_tensor_tensor
                stt(mu[4], sy, mk, mu[0], ALU.bitwise_and, ALU.bitwise_xor)
                stt(mu[2], sx, mk, mu[0], ALU.bitwise_and, ALU.bitwise_xor)
                stt(mu[1], sd, mk, mu[0], ALU.bitwise_and, ALU.bitwise_xor)
                stt(mu[6], sx, mk, mu[4], ALU.bitwise_and, ALU.bitwise_xor)
                stt(mu[5], sd, mk, mu[4], ALU.bitwise_and, ALU.bitwise_xor)
                stt(mu[3], sd, mk, mu[2], ALU.bitwise_and, ALU.bitwise_xor)
                stt(mu[7], sd, mk, mu[6], ALU.bitwise_and, ALU.bitwise_xor)

                if debug_chunk == ci:
                    nc.sync.dma_start(dbg_mono[:], mono[:])
                    nc.sync.dma_start(dbg_g[:, 0], gxh[:])
                    nc.sync.dma_start(dbg_g[:, 1], gyh[:])
                    nc.sync.dma_start(dbg_g[:, 2], eh[:])
                    nc.sync.dma_start(dbg_x[:], xt[:])

                # ---- conv matmuls: per 4-row slot, 4-way PE tile packing
                for sj in range(R // 4):
                    ps_t = ppool.tile([128, 512], F32, tag="ps_t")
                    ps_b = ppool.tile([128, 512], F32, tag="ps_b")
                    first = True
                    for m in range(8):
                        for tap in range(9):
                            dy, dx = tap // 3, tap % 3
                            rA = 4 * sj + dy
                            rB = rA + 2
                            st = (m == 7 and tap == 8)
                            for (pr, ps, rr) in ((0, ps_t, rA), (64, ps_b, rA),
                                                 (0, ps_t, rB), (64, ps_b, rB)):
                                pc = 0 if rr == rA else 64
                                nc.tensor.matmul(
                                    ps[pc:pc + 64, :],
                                    wt[pr:pr + 64, m, tap, :],
                                    mono[pr:pr + 64, m, rr:rr + 2, dx:dx + W_],
                                    start=first, stop=st,
                                    skip_group_check=True,
                                )
                            first = False
                    # ---- evacuate PSUM (+bias) and store
                    y0 = r0 + 4 * sj
                    stg_t = spool.tile([128, 512], F32, tag="stg")
                    nc.scalar.activation(stg_t[:], ps_t[:], ACTF.Identity,
                                         bias=biasT[:, 0:1])
                    stg_b = spool.tile([128, 512], F32, tag="stg")
                    nc.scalar.activation(stg_b[:], ps_b[:], ACTF.Identity,
                                         bias=biasT[:, 0:1])
                    nc.sync.dma_start(out_d[:, y0:y0 + 2, :], stg_t[0:64])
                    nc.sync.dma_start(out_d[:, y0 + 2:y0 + 4, :], stg_t[64:128])
                    yb = HH + y0
                    nc.sync.dma_start(out_d[:, yb:yb + 2, :], stg_b[0:64])
                    nc.sync.dma_start(out_d[:, yb + 2:yb + 4, :], stg_b[64:128])

    nc.compile()
    return nc


def _prep_host_inputs(Wfull: np.ndarray, bfull: np.ndarray):
    """Monomial weights wt[128, 8, 9, O] fp16 and bias[128,1] fp32."""
    sig = np.zeros((K, 3), np.float64)
    for k in range(K):
        a_, b_, c_ = (k >> 2) & 1, (k >> 1) & 1, k & 1
        Sy, Sx, D = a_, a_ ^ b_, b_ ^ c_
        sig[k] = [2 * Sy - 1, 2 * Sx - 1, 2 * D - 1]
    Wd = Wfull.astype(np.float64)  # (K, O, C, 3, 3)
    wt = np.zeros((64, 8, 9, O), np.float64)
    for S in range(8):
        coef = np.ones(K)
        if S & 4: coef = coef * sig[:, 0]
        if S & 2: coef = coef * sig[:, 1]
        if S & 1: coef = coef * sig[:, 2]
        Wp = np.einsum('k,kocyx->ocyx', coef, Wd) / 64.0  # (O, C, 3, 3)
        wt[:, S, :, :] = np.transpose(Wp.reshape(O, C, 9), (1, 2, 0))
    wt128 = np.concatenate([wt, wt], axis=0).astype(np.float16)
    bias = (bfull.astype(np.float64).sum(axis=0) / K).astype(np.float32)
    bias128 = np.concatenate([bias, bias])[:, None]
    return wt128, bias128


_NC_CACHE = None


def _get_nc():
    global _NC_CACHE
    if _NC_CACHE is None:
        _NC_CACHE = _build_nc()
    return _NC_CACHE


LAST_RESULT = None


def kernel(x: np.ndarray, W: np.ndarray, b: np.ndarray, **run_kwargs) -> np.ndarray:
    global LAST_RESULT
    assert x.shape == (B, C, H, W_) and W.shape == (K, O, C, 3, 3)
    nc = _get_nc()
    wt128, bias128 = _prep_host_inputs(np.asarray(W), np.asarray(b))
    xs = np.ascontiguousarray(np.asarray(x, dtype=np.float32))
    in_maps = [
        {"x": xs[i], "wt": wt128, "bias": bias128}
        for i in range(B)
    ]
    res = bass_utils.run_bass_kernel_spmd(nc, in_maps, core_ids=list(range(B)),
                                          **run_kwargs)
    LAST_RESULT = res
    out = np.stack([res.results[i]["out"] for i in range(B)], axis=0)
    return out.astype(np.float32)


if __name__ == "__main__":
    nc = _get_nc()
    print("built + compiled OK")


# revision 11
# speedup vs baseline: 1.0306x; 1.0306x over previous
"""Trainium2 Bass kernel for nn_DirectionalConv (moe_routing).

Math: out = (1/8) * sum_k conv3x3(x * [octant(sobel(x)) == k], W[k]) + mean_k b[k]

Implementation notes:
- Data-parallel over batch B=8 across 8 NeuronCores (one image per core).
- Octant selection is rewritten in a +-1 "monomial" basis over the three sign
  bits (sign(gy), sign(gx), sign(|gy|-|gx|)):
      sum_k conv(x*mask_k, W[k]) = sum_{S in 2^3} conv(x*chi_S, W'_S)
  where chi_S = product of the selected signs (computed with pure bitwise
  XOR of sign bits - exact) and W'_S = (1/64) sum_k chi_S(k) W[k] is
  precomputed on the host.  This gives 8 dense 3x3 convs, evaluated as
  9 shifted matmuls each, accumulating in PSUM.
- Per-core image (64,256,256) is split into top/bottom halves across the
  SBUF partition dimension: partition p = (half<<6)|channel.  Conv matmuls
  are K=64 and use 4-way PE tile packing (2 row-groups x 2 col-groups) to
  fill the 128x128 array.
- Sobel gradients are computed in fp32 on the vector engine (separable,
  scalar_tensor_tensor fused ops); monomials/weights are fp16 (rel err ~1e-3).
"""

import numpy as np

import concourse.bacc as bacc
import concourse.bass as bass
import concourse.mybir as mybir
from concourse import bass_utils
from concourse.tile import TileContext

F32 = mybir.dt.float32
F16 = mybir.dt.float16
U32 = mybir.dt.uint32
ALU = mybir.AluOpType
ACTF = mybir.ActivationFunctionType

B, C, H, W_, K, O = 8, 64, 256, 256, 8, 64
HH = H // 2          # rows per half
R = 8                # output rows per half per chunk
NCHUNK = HH // R     # 16
WP = W_ + 2          # padded width 258
SIGN16 = 0x80008000  # sign bits of two packed fp16 lanes


def _build_nc(debug_chunk=None):
    nc = bacc.Bacc("TRN2", target_bir_lowering=False, debug=False)

    x_d = nc.dram_tensor("x", [C, H, W_], F32, kind="ExternalInput")
    wt_d = nc.dram_tensor("wt", [128, 8, 9, O], F16, kind="ExternalInput")
    bias_d = nc.dram_tensor("bias", [128, 1], F32, kind="ExternalInput")
    out_d = nc.dram_tensor("out", [O, H, W_], F32, kind="ExternalOutput")
    if debug_chunk is not None:
        dbg_mono = nc.dram_tensor("dbg_mono", [128, 8, R + 2, WP], F16,
                                  kind="ExternalOutput")
        dbg_g = nc.dram_tensor("dbg_g", [128, 3, R + 2, WP], F16,
                               kind="ExternalOutput")
        dbg_x = nc.dram_tensor("dbg_x", [128, R + 4, WP], F32,
                               kind="ExternalOutput")

    with TileContext(nc) as tc:
        with (
            tc.tile_pool(name="wpool", bufs=1) as wpool,
            tc.tile_pool(name="xpool", bufs=2) as xpool,
            tc.tile_pool(name="tpool", bufs=1) as tpool,
            tc.tile_pool(name="mpool", bufs=2) as mpool,
            tc.tile_pool(name="spool", bufs=4) as spool,
            tc.tile_pool(name="ppool", bufs=3, space="PSUM") as ppool,
        ):
            wt = wpool.tile([128, 8, 9, O], F16)
            nc.sync.dma_start(wt[:], wt_d[:])
            biasT = wpool.tile([128, 1], F32)
            nc.sync.dma_start(biasT[:], bias_d[:])
            maskT = wpool.tile([128, 1], U32)
            nc.gpsimd.memset(maskT[:], SIGN16)

            for ci in range(NCHUNK):
                r0 = ci * R
                # ---- load x chunk (rows r0-2 .. r0+R+2 per half), cols 1..256
                xt = xpool.tile([128, R + 4, WP], F32, tag="xt")
                nc.gpsimd.memset(xt[:, :, 0:1], 0.0)
                nc.gpsimd.memset(xt[:, :, WP - 1:WP], 0.0)
                # top half -> partitions 0..63
                tlo, thi = r0 - 2, r0 + R + 2
                if tlo < 0:
                    nc.gpsimd.memset(xt[0:64, 0:-tlo, 1:WP - 1], 0.0)
                    nc.sync.dma_start(xt[0:64, -tlo:R + 4, 1:WP - 1],
                                      x_d[:, 0:thi, :])
                else:
                    nc.sync.dma_start(xt[0:64, :, 1:WP - 1], x_d[:, tlo:thi, :])
                # bottom half -> partitions 64..127
                blo, bhi = HH + r0 - 2, HH + r0 + R + 2
                if bhi > H:
                    nval = H - blo
                    nc.gpsimd.memset(xt[64:128, nval:R + 4, 1:WP - 1], 0.0)
                    nc.sync.dma_start(xt[64:128, 0:nval, 1:WP - 1],
                                      x_d[:, blo:H, :])
                else:
                    nc.sync.dma_start(xt[64:128, :, 1:WP - 1], x_d[:, blo:bhi, :])

                # ---- monomial 0 (= fp16 cast of x) first, on the scalar
                # engine: lets the tensor engine start m=0 matmuls while the
                # vector engine computes gradients.
                RG = R + 2
                mono = mpool.tile([128, 8, RG, WP], F16, tag="mono")
                nc.scalar.activation(mono[:, 0], xt[:, 1:RG + 1, :], ACTF.Copy)

                # ---- Sobel gradients (fp32, separable) on rows r0-1..r0+R
                at = tpool.tile([128, RG, WP], F32, tag="at")
                nc.vector.tensor_add(at[:], xt[:, 0:RG, :], xt[:, 2:RG + 2, :])
                tt = tpool.tile([128, RG, WP], F32, tag="tt")
                nc.vector.scalar_tensor_tensor(tt[:], xt[:, 1:RG + 1, :], 2.0,
                                               at[:], ALU.mult, ALU.add)
                ut = tpool.tile([128, RG, WP], F32, tag="ut")
                nc.vector.tensor_sub(ut[:], xt[:, 0:RG, :], xt[:, 2:RG + 2, :])
                gx32 = tpool.tile([128, RG, WP], F32, tag="gx32")
                nc.gpsimd.memset(gx32[:, :, 0:1], 0.0)
                nc.gpsimd.memset(gx32[:, :, WP - 1:WP], 0.0)
                nc.vector.tensor_tensor(gx32[:, :, 1:WP - 1], tt[:, :, 0:WP - 2],
                                        tt[:, :, 2:WP], ALU.subtract)
                b2 = tpool.tile([128, RG, W_], F32, tag="at")  # reuse at's slot
                nc.vector.tensor_add(b2[:], ut[:, :, 0:WP - 2], ut[:, :, 2:WP])
                gy32 = tpool.tile([128, RG, WP], F32, tag="gy32")
                nc.gpsimd.memset(gy32[:, :, 0:1], 0.0)
                nc.gpsimd.memset(gy32[:, :, WP - 1:WP], 0.0)
                nc.vector.scalar_tensor_tensor(gy32[:, :, 1:WP - 1],
                                               ut[:, :, 1:WP - 1], 2.0, b2[:],
                                               ALU.mult, ALU.add)
                # fp16 sign sources (casts on the scalar engine, which is idle)
                gxh = tpool.tile([128, RG, WP], F16, tag="gxh")
                nc.scalar.activation(gxh[:], gx32[:], ACTF.Copy)
                gyh = tpool.tile([128, RG, WP], F16, tag="gyh")
                nc.scalar.activation(gyh[:], gy32[:], ACTF.Copy)
                # e = |gy|-|gx| in fp32 (only its sign is used; fp16 rounding
                # of the comparison would misbin ~1e-4 of pixels -> 1.5e-2 err)
                ay = tpool.tile([128, RG, W_], F32, tag="tt")
                nc.scalar.activation(ay[:], gy32[:, :, 1:WP - 1], ACTF.Abs)
                ax = tpool.tile([128, RG, W_], F32, tag="ut")
                nc.scalar.activation(ax[:], gx32[:, :, 1:WP - 1], ACTF.Abs)
                e32 = tpool.tile([128, RG, WP], F32, tag="e32")
                nc.gpsimd.memset(e32[:, :, 0:1], 0.0)
                nc.gpsimd.memset(e32[:, :, WP - 1:WP], 0.0)
                nc.vector.tensor_tensor(e32[:, :, 1:WP - 1], ay[:], ax[:],
                                        ALU.subtract)
                eh = tpool.tile([128, RG, WP], F16, tag="eh")
                nc.scalar.activation(eh[:], e32[:], ACTF.Copy)

                # ---- monomials y_S = x * chi_S as fp16, S = (sy<<2)|(sx<<1)|sd
                mu = [mono[:, S].bitcast(U32) for S in range(8)]
                sy = gyh[:].bitcast(U32)
                sx = gxh[:].bitcast(U32)
                sd = eh[:].bitcast(U32)
                mk = maskT[:, 0:1]
                stt = nc.vector.scalar_tensor_tensor
                stt(mu[4], sy, mk, mu[0], ALU.bitwise_and, ALU.bitwise_xor)
                stt(mu[2], sx, mk, mu[0], ALU.bitwise_and, ALU.bitwise_xor)
                stt(mu[1], sd, mk, mu[0], ALU.bitwise_and, ALU.bitwise_xor)
                stt(mu[6], sx, mk, mu[4], ALU.bitwise_and, ALU.bitwise_xor)
                stt(mu[5], sd, mk, mu[4], ALU.bitwise_and, ALU.bitwise_xor)
                stt(mu[3], sd, mk, mu[2], ALU.bitwise_and, ALU.bitwise_xor)
                stt(mu[7], sd, mk, mu[6], ALU.bitwise_and, ALU.bitwise_xor)

                if debug_chunk == ci:
                    nc.sync.dma_start(dbg_mono[:], mono[:])
                    nc.sync.dma_start(dbg_g[:, 0], gxh[:])
                    nc.sync.dma_start(dbg_g[:, 1], gyh[:])
                    nc.sync.dma_start(dbg_g[:, 2], eh[:])
                    nc.sync.dma_start(dbg_x[:], xt[:])

                # ---- conv matmuls: per 4-row slot, 4-way PE tile packing
                for sj in range(R // 4):
                    ps_t = ppool.tile([128, 512], F32, tag="ps_t")
                    ps_b = ppool.tile([128, 512], F32, tag="ps_b")
                    first = True
                    for m in (0, 4, 2, 1, 6, 5, 3, 7):  # DVE completion order
                        for tap in range(9):
                            dy, dx = tap // 3, tap % 3
                            rA = 4 * sj + dy
                            rB = rA + 2
                            st = (m == 7 and tap == 8)
                            for (pr, ps, rr) in ((0, ps_t, rA), (64, ps_b, rA),
                                                 (0, ps_t, rB), (64, ps_b, rB)):
                                pc = 0 if rr == rA else 64
                                nc.tensor.matmul(
                                    ps[pc:pc + 64, :],
                                    wt[pr:pr + 64, m, tap, :],
                                    mono[pr:pr + 64, m, rr:rr + 2, dx:dx + W_],
                                    start=first, stop=st,
                                    skip_group_check=True,
                                )
                            first = False
                    # ---- evacuate PSUM (+bias) and store
                    y0 = r0 + 4 * sj
                    stg_t = spool.tile([128, 512], F32, tag="stg")
                    nc.scalar.activation(stg_t[:], ps_t[:], ACTF.Identity,
                                         bias=biasT[:, 0:1])
                    stg_b = spool.tile([128, 512], F32, tag="stg")
                    nc.scalar.activation(stg_b[:], ps_b[:], ACTF.Identity,
                                         bias=biasT[:, 0:1])
                    nc.sync.dma_start(out_d[:, y0:y0 + 2, :], stg_t[0:64])
                    nc.sync.dma_start(out_d[:, y0 + 2:y0 + 4, :], stg_t[64:128])
                    yb = HH + y0
                    nc.sync.dma_start(out_d[:, yb:yb + 2, :], stg_b[0:64])
                    nc.sync.dma_start(out_d[:, yb + 2:yb + 4, :], stg_b[64:128])

    nc.compile()
    return nc


def _prep_host_inputs(Wfull: np.ndarray, bfull: np.ndarray):
    """Monomial weights wt[128, 8, 9, O] fp16 and bias[128,1] fp32."""
    sig = np.zeros((K, 3), np.float64)
    for k in range(K):
        a_, b_, c_ = (k >> 2) & 1, (k >> 1) & 1, k & 1
        Sy, Sx, D = a_, a_ ^ b_, b_ ^ c_
        sig[k] = [2 * Sy - 1, 2 * Sx - 1, 2 * D - 1]
    Wd = Wfull.astype(np.float64)  # (K, O, C, 3, 3)
    wt = np.zeros((64, 8, 9, O), np.float64)
    for S in range(8):
        coef = np.ones(K)
        if S & 4: coef = coef * sig[:, 0]
        if S & 2: coef = coef * sig[:, 1]
        if S & 1: coef = coef * sig[:, 2]
        Wp = np.einsum('k,kocyx->ocyx', coef, Wd) / 64.0  # (O, C, 3, 3)
        wt[:, S, :, :] = np.transpose(Wp.reshape(O, C, 9), (1, 2, 0))
    wt128 = np.concatenate([wt, wt], axis=0).astype(np.float16)
    bias = (bfull.astype(np.float64).sum(axis=0) / K).astype(np.float32)
    bias128 = np.concatenate([bias, bias])[:, None]
    return wt128, bias128


_NC_CACHE = None


def _get_nc():
    global _NC_CACHE
    if _NC_CACHE is None:
        _NC_CACHE = _build_nc()
    return _NC_CACHE


LAST_RESULT = None


def kernel(x: np.ndarray, W: np.ndarray, b: np.ndarray, **run_kwargs) -> np.ndarray:
    global LAST_RESULT
    assert x.shape == (B, C, H, W_) and W.shape == (K, O, C, 3, 3)
    nc = _get_nc()
    wt128, bias128 = _prep_host_inputs(np.asarray(W), np.asarray(b))
    xs = np.ascontiguousarray(np.asarray(x, dtype=np.float32))
    in_maps = [
        {"x": xs[i], "wt": wt128, "bias": bias128}
        for i in range(B)
    ]
    res = bass_utils.run_bass_kernel_spmd(nc, in_maps, core_ids=list(range(B)),
                                          **run_kwargs)
    LAST_RESULT = res
    out = np.stack([res.results[i]["out"] for i in range(B)], axis=0)
    return out.astype(np.float32)


if __name__ == "__main__":
    nc = _get_nc()
    print("built + compiled OK")
